# revision 74
# baseline (speedup 1.0000x reference)
"""Trainium2 Bass kernel for nn_Attention_80384607912675.

Multi-head attention (B=2, S=2048, D=1024, H=16, HD=64), fp32 reference.

Sharding (8 cores): data-parallel over batch (2) x tensor-parallel over heads
(4 head groups of 4 heads).  Core c handles batch c//4, heads [4*(c%4), ...).
wq/wk/wv split column-wise, wo split row-wise; the 4 per-batch partials are
summed on the host (+bo).

Key structure (timings per the TRN2 cost model, matmul cost = N_free x rate):
  - QKV projections in fp8e4 DoubleRow (2 k-tiles/instr, 0.5 cyc/row) with an
    exact-to-~0.15% hi/lo split of x and w (split on host; 3 of 4 product
    terms kept).
  - Scores S^T = K^T (x) Q via a 256-deep augmented contraction
    [q_hi;q_lo]x2 * [[k_hi;k_hi],[k_lo;k_lo]] in ONE fp8 DoubleRow instr per
    (head, kp-chunk): exact product reconstruction in half the bf16 PE time.
    The aug layouts are built by partition-remap DMAs from the drain outputs.
  - exp on ScalarE ([128,1024] tiles straight from PSUM; the Schraudolph
    DVE-offload machinery exists but is disabled - it measured slower).
  - PV flipped: O[q, d] = sum_c P^T-chunk.T @ V_aug-chunk, N=65 per matmul
    (instead of 512): half the PE time.  V_aug's ones column gives the
    softmax rowsum in column 64 of each oacc sub-tile.
  - O normalized per (head, q-block) via tensor_scalar with per-partition
    reciprocal, PE-transposed to O^T for the wo matmul.
  - Output projection accumulates both head-pairs in PSUM (single out DRAM
    tensor per core).
  - Each chunk's QK is hoisted one slot ahead of its exp and PV lags by 3
    chunks, so the ScalarE exp stream is never gated by PE fillers;
    projections/outproj/transposes are spread as static fillers.
  - Weights are host-scaled by 32 into e4m3's normal range (compensated in
    the exp scale and wo); pair-0 q/k stay bf16, pair-1 uses the fp8 path.
"""

import numpy as np

B, S, D, H = 2, 2048, 1024, 16
HD = D // H          # 64
HPC = 4              # heads per core
DHC = HPC * HD       # 256 head dims per core
KC = D // 128        # 8 contraction chunks of 128
NCH = S // 128       # 16 kp chunks / s blocks / q blocks
VP = HPC * (HD + 1)  # 260: V_aug pitch per s-chunk
NC = 8               # cores
NQW = 4              # 512-wide q windows per head pair
NW = 2 * NQW         # 8 windows

# oacc sub-tile col offsets (f32) for j = qb*2 + head_i; j=7 starts at bank 1
OFFS = [0, 65, 130, 195, 260, 325, 390, 512]

# (window, c) exp tiles computed on DVE+Pool instead of ScalarE; placed in
# windows with DVE slack, away from the slot-0/1 norm drains, so step-1 runs
# promptly and the sc slot frees before the hoisted QK two chunks later
SCHR = set()  # Schraudolph offload measured slower with the final schedule
# pair-0 windows (0..3) run bf16 QK: no fp8 aug prep deadlines on the lead /
# early windows; pair-1 windows (4..7) use the fp8 DoubleRow aug path whose
# prep has three windows of runway.
_LOG2E = 1.4426950408889634
# wq/wk/wv are pre-scaled by WSC on the host so their values (and the hi/lo
# fp8 split residuals) sit in e4m3's normal range; scores then carry a
# WSC^2 factor that is absorbed into the exp scale, and V's WSC into wo.
WSC = 32.0
EXPSC = 0.125 / (WSC * WSC)
SCHR_A = EXPSC * _LOG2E * (1 << 23)
SCHR_B = float(127 * (1 << 23)) - 366000.0

WNAMES = ("qh", "ql", "kh", "kl", "vh", "vl")

_nc_cache = {}


def _build_bass(with_bias=False, debug=False):
    import concourse.mybir as mybir
    import concourse.tile as tile
    from concourse import bacc

    BF = mybir.dt.bfloat16
    F32 = mybir.dt.float32
    FP8 = mybir.dt.float8e4
    I32 = mybir.dt.int32
    EXP = mybir.ActivationFunctionType.Exp
    DR = mybir.MatmulPerfMode.DoubleRow
    MUL = mybir.AluOpType.mult
    ADD = mybir.AluOpType.add
    SUB = mybir.AluOpType.subtract

    nc = bacc.Bacc("TRN2")

    xh_d = nc.dram_tensor("xh", [D, S], FP8, kind="ExternalInput")
    xl_d = nc.dram_tensor("xl", [D, S], FP8, kind="ExternalInput")
    w8_d = nc.dram_tensor("w8", [128, 6 * KC * DHC], FP8, kind="ExternalInput")
    wo_d = nc.dram_tensor("wo_c", [DHC, D], BF, kind="ExternalInput")
    if with_bias:
        bias_d = nc.dram_tensor("bias3", [1, 3 * DHC], BF, kind="ExternalInput")
    out_d = nc.dram_tensor("out", [S, D], BF, kind="ExternalOutput")
    if debug:
        dbg = {
            "v": nc.dram_tensor("dbg_v", [128, NCH * VP], BF,
                                kind="ExternalOutput"),
            "ktbf": nc.dram_tensor("dbg_ktbf", [128, 2048], BF,
                                   kind="ExternalOutput"),
            "qtbf": nc.dram_tensor("dbg_qtbf", [128, NQW * 512], BF,
                                   kind="ExternalOutput"),
            "qaug": nc.dram_tensor("dbg_qaug", [128, HPC * S],
                                   mybir.dt.float8e4, kind="ExternalOutput"),
            "kaug": nc.dram_tensor("dbg_kaug", [128, 2 * HPC * S],
                                   mybir.dt.float8e4, kind="ExternalOutput"),
            "onT": nc.dram_tensor("dbg_onT", [128, 2 * S], BF,
                                  kind="ExternalOutput"),
            "oacc": nc.dram_tensor("dbg_oacc", [128, 1024], mybir.dt.float32,
                                   kind="ExternalOutput"),
            "r": nc.dram_tensor("dbg_r", [128, 8], mybir.dt.float32,
                                kind="ExternalOutput"),
            "onm": nc.dram_tensor("dbg_onm", [128, 4 * 128], BF,
                                  kind="ExternalOutput"),
        }

    with tile.TileContext(nc) as tc:
        with (
            tc.tile_pool(name="persist", bufs=1) as pp,
            tc.tile_pool(name="sc", bufs=2, space="PSUM") as scp,
            tc.tile_pool(name="oacc", bufs=1, space="PSUM") as opp,
            tc.tile_pool(name="pj", bufs=2, space="PSUM") as pjp,
            tc.tile_pool(name="pt", bufs=6) as ptp,
            tc.tile_pool(name="rc", bufs=2) as rcp,
            tc.tile_pool(name="onm", bufs=4) as onp,
            tc.tile_pool(name="schr", bufs=2) as shp,
            tc.tile_pool(name="osb", bufs=6) as oup,
        ):
            xh_sb = pp.tile([128, KC, S], FP8, tag="xh", name="xh_sb")
            xl_sb = pp.tile([128, KC, S], FP8, tag="xl", name="xl_sb")
            w8 = pp.tile([128, 6, KC, DHC], FP8, tag="w8", name="w8_sb")
            w_sb = {nm: w8[:, i] for i, nm in enumerate(WNAMES)}
            wo_sb = pp.tile([128, 2, D], BF, tag="wo", name="wo_sb")
            # drain targets: [128 rows = head-pair dims, hi/lo, p-block*S + s]
            qt8 = pp.tile([128, 2, 2 * S], FP8, tag="qt8", name="qt8")
            kt8 = pp.tile([128, 2, 2 * S], FP8, tag="kt8", name="kt8")
            # augmented QK layouts
            #   qaug: per head h cols h*S+s, rows [q_hi(64); q_lo(64)]
            #   kaug: per (head h, ktile t) cols (2h+t)*S+s, rows [k_x; k_x]
            qaug = pp.tile([128, HPC * S], FP8, tag="qaug", name="qaug")
            kaug = pp.tile([128, 2 * HPC * S], FP8, tag="kaug", name="kaug")
            # bf16 q/k for the pair-0 windows
            qt_bf = pp.tile([128, NQW * 512], BF, tag="qt_bf", name="qt_bf")
            kt_bf = pp.tile([128, 2048], BF, tag="kt_bf", name="kt_bf")
            v_sb = pp.tile([128, NCH * VP], BF, tag="v", name="v_sb")
            onT = pp.tile([128, 2 * S], BF, tag="onT", name="onT")
            ident = pp.tile([128, 128], BF, tag="ident", name="ident")
            if with_bias:
                bias_sb = pp.tile([1, 3 * DHC], BF, tag="bias", name="bias_sb")
                ones1 = pp.tile([1, 512], BF, tag="ones1", name="ones1")

            # --- input DMAs: weights (one fp8 blob), xh chunks, xl, wo last
            nc.sync.dma_start(
                w8[:, :, :, :].rearrange("p a b c -> p (a b c)"), w8_d[:, :]
            )
            if with_bias:
                nc.sync.dma_start(bias_sb[:, :], bias_d[:, :])
                nc.vector.memset(ones1[:, :], 1.0)
            for k in range(KC):
                nc.sync.dma_start(xh_sb[:, k, :], xh_d[k * 128:(k + 1) * 128, :])
            for k in range(KC):
                nc.sync.dma_start(xl_sb[:, k, :], xl_d[k * 128:(k + 1) * 128, :])
            nc.sync.dma_start(
                wo_sb[:, :, :], wo_d[:, :].rearrange("(r p) d -> p r d", p=128)
            )

            from concourse.masks import make_identity
            make_identity(nc, ident[:, :])
            nc.gpsimd.memset(v_sb[:, :], 1.0)  # ones cols of V_aug

            # PE warmup: keep the tensor engine busy through the x-DMA
            # stream so it reaches full p-state before the lead matmuls
            warm = pjp.tile([128, 128], BF, tag="pj", name="warm")
            for _ in range(40):
                nc.tensor.transpose(warm[:, :], ident[:, :], ident[:, :])

            # --- fp8 3-term projection helpers -----------------------------
            def qk_mm(ps, hi_nm, lo_nm, p, nt, term, cp):
                """term 0: xh*wh, 1: xh*wl, 2: xl*wh (DoubleRow chunk-pair)."""
                wsb = w_sb[hi_nm if term != 1 else lo_nm]
                xsb = xh_sb if term != 2 else xl_sb
                nc.tensor.matmul(
                    ps[:, :],
                    lhsT=wsb[:, 2 * cp:2 * cp + 2, p * 128:(p + 1) * 128],
                    rhs=xsb[:, 2 * cp:2 * cp + 2, nt * 512:(nt + 1) * 512],
                    start=(term == 0 and cp == 0),
                    stop=(not with_bias and term == 2 and cp == 3),
                    perf_mode=DR,
                )

            def qk_bias(ps, boff, p):
                nc.tensor.matmul(
                    ps[:, :],
                    lhsT=bias_sb[:, boff + p * 128: boff + (p + 1) * 128],
                    rhs=ones1[0:1, :],
                    start=False,
                    stop=True,
                )

            def qk_drain(ps, dst, p, nt):
                """hi/lo fp8 split of a projected [128,512] tile."""
                cols = slice(p * S + nt * 512, p * S + (nt + 1) * 512)
                nc.vector.tensor_copy(dst[:, 0, cols], ps[:, :])
                nc.vector.scalar_tensor_tensor(
                    out=dst[:, 1, cols], in0=ps[:, :], scalar=1.0,
                    in1=dst[:, 0, cols], op0=MUL, op1=SUB,
                )

            _half = {}

            def proj_qk_a(dst, hi_nm, lo_nm, p, nt):
                ps = pjp.tile(
                    [128, 512], F32, tag="pj", name=f"pj_{hi_nm}{p}{nt}"
                )
                for term in range(2):
                    for cp in range(4):
                        qk_mm(ps, hi_nm, lo_nm, p, nt, term, cp)
                _half[(hi_nm, p, nt)] = ps

            def proj_qk_b(dst, hi_nm, lo_nm, boff, p, nt):
                """dst None -> bf16 drain into qt_bf window slot nt."""
                ps = _half.pop((hi_nm, p, nt))
                for cp in range(4):
                    qk_mm(ps, hi_nm, lo_nm, p, nt, 2, cp)
                if with_bias:
                    qk_bias(ps, boff, p)
                if dst is None:
                    nc.vector.tensor_copy(
                        qt_bf[:, nt * 512:(nt + 1) * 512], ps[:, :]
                    )
                else:
                    qk_drain(ps, dst, p, nt)

            def proj_v(sb):
                """V_aug s-block sb in natural layout [128 s, 256 d]."""
                ps = pjp.tile([128, 256], F32, tag="pj", name=f"pv_{sb}")
                for term in range(3):
                    wsb = w_sb["vh" if term != 1 else "vl"]
                    xsb = xh_sb if term != 2 else xl_sb
                    for cp in range(4):
                        nc.tensor.matmul(
                            ps[:, :],
                            lhsT=xsb[:, 2 * cp:2 * cp + 2,
                                     sb * 128:(sb + 1) * 128],
                            rhs=wsb[:, 2 * cp:2 * cp + 2, :],
                            start=(term == 0 and cp == 0),
                            stop=(not with_bias and term == 2 and cp == 3),
                            perf_mode=DR,
                        )
                if with_bias:
                    nc.tensor.matmul(
                        ps[:, :],
                        lhsT=ones1[0:1, 0:128],
                        rhs=bias_sb[:, 2 * DHC:3 * DHC],
                        start=False,
                        stop=True,
                    )
                dst3 = v_sb[:, sb * VP:(sb + 1) * VP].rearrange(
                    "p (h e) -> p h e", e=HD + 1
                )[:, :, 0:HD]
                nc.vector.tensor_copy(dst3, ps[:, :])

            # --- aug-layout remap DMAs ------------------------------------
            def remap_k(p, nt_lo, nt_hi, eng=None):
                """kaug[(h,t) cols] <- kt8: one DMA per (head, half)."""
                eng = eng or nc.sync
                c0, c1 = nt_lo * 512, nt_hi * 512
                w = c1 - c0
                for i in range(2):
                    h = 2 * p + i
                    src = kt8[64 * i:64 * (i + 1), :, p * S + c0: p * S + c1]
                    dst = kaug[:, 2 * h * S: 2 * (h + 1) * S].rearrange(
                        "p (t s) -> p t s", t=2
                    )[:, :, c0:c0 + w]
                    for half in range(2):
                        eng.dma_start(dst[64 * half:64 * half + 64, :, :], src)

            def remap_q(p, nt_lo, nt_hi, eng=None):
                """qaug rows [q_hi; q_lo] per head: one DMA per (head, t)."""
                eng = eng or nc.sync
                c0, c1 = nt_lo * 512, nt_hi * 512
                for i in range(2):
                    h = 2 * p + i
                    for t in range(2):
                        eng.dma_start(
                            qaug[64 * t:64 * t + 64, h * S + c0: h * S + c1],
                            qt8[64 * i:64 * (i + 1), t, p * S + c0: p * S + c1],
                        )

            # --- output projection (both pairs accumulated) ---------------
            def outproj(sb, on_act=False):
                ot = oup.tile([128, 1024], BF, tag="osb", name=f"ot_{sb}")
                for n in range(2):
                    po = pjp.tile([128, 512], F32, tag="pj", name=f"po_{sb}_{n}")
                    for hp in range(2):
                        nc.tensor.matmul(
                            po[:, :],
                            lhsT=onT[:, hp * S + sb * 128: hp * S + (sb + 1) * 128],
                            rhs=wo_sb[:, hp, n * 512:(n + 1) * 512],
                            start=(hp == 0),
                            stop=(hp == 1),
                        )
                    if on_act and n == 1:
                        nc.scalar.copy(ot[:, n * 512:(n + 1) * 512], po[:, :])
                    else:
                        nc.vector.tensor_copy(
                            ot[:, n * 512:(n + 1) * 512], po[:, :]
                        )
                nc.sync.dma_start(
                    out_d[sb * 128:(sb + 1) * 128, :], ot[:, :]
                )

            # --- window drain: normalize O, transpose to O^T --------------
            def drain_recip(hp, qw, oacc):
                rs = rcp.tile([128, 8], F32, tag="rs", name=f"rs_{hp}{qw}")
                nc.vector.tensor_copy(
                    rs[:, 0:7].rearrange("p (j e) -> p j e", e=1),
                    oacc[:, 0:7 * 65].rearrange(
                        "p (j e) -> p j e", e=65)[:, :, 64:65],
                )
                nc.vector.tensor_copy(
                    rs[:, 7:8], oacc[:, OFFS[7] + 64:OFFS[7] + 65]
                )
                r = rcp.tile([128, 8], F32, tag="rc", name=f"rc_{hp}{qw}")
                nc.vector.reciprocal_approx_fast(out=r[:, :], in_=rs[:, :])
                return r

            def drain_norm(hp, qw, oacc, r, qb, on_act=False):
                onm = onp.tile([128, 128], BF, tag="onm", name=f"on_{hp}{qw}{qb}")
                for i in range(2):
                    j = qb * 2 + i
                    if on_act and i == 1:
                        nc.scalar.activation(
                            onm[:, 64 * i:64 * i + 64],
                            oacc[:, OFFS[j]:OFFS[j] + 64],
                            mybir.ActivationFunctionType.Copy,
                            scale=r[:, j:j + 1],
                        )
                    else:
                        nc.vector.tensor_scalar(
                            out=onm[:, 64 * i:64 * i + 64],
                            in0=oacc[:, OFFS[j]:OFFS[j] + 64],
                            scalar1=r[:, j:j + 1],
                            scalar2=None,
                            op0=MUL,
                        )
                return onm

            def drain_tp(hp, qw, qb, onm):
                tp = pjp.tile([128, 128], BF, tag="pj", name=f"tp_{hp}{qw}{qb}")
                nc.tensor.transpose(tp[:, :], onm[:, :], ident[:, :])
                nc.vector.tensor_copy(
                    onT[:, hp * S + qw * 512 + qb * 128:
                        hp * S + qw * 512 + (qb + 1) * 128],
                    tp[:, :],
                )

            # --- lead-in: k-proj p0 nt0/nt1 + q-proj p0 nt0 (3 PSUM tiles,
            # keeping the sc ring free so QK(0,0) can issue immediately).
            # Window 0 runs fully on bf16 q/k (NBF=16); k-nt2/nt3 projections
            # run as early window-0 fillers.
            lead = [
                ("k", kt8, 0, 0, pjp, "pj"), ("k", kt8, 0, 1, pjp, "pj"),
                ("q", qt8, 0, 0, opp, "oacc"),
            ]
            lead_ps = [
                pool.tile([128, 512], F32, tag=tag, name=f"lead_{w}{p}{nt}")
                for w, dst, p, nt, pool, tag in lead
            ]
            for term in range(3):
                for cp in range(4):
                    for (w, dst, p, nt, pool, tag), ps in zip(lead, lead_ps):
                        qk_mm(ps, f"{w}h", f"{w}l", p, nt, term, cp)
            if with_bias:
                for li in range(3):
                    w, dst, p, nt, pool, tag = lead[li]
                    qk_bias(lead_ps[li], 0 if w == "q" else DHC, p)
            with tc.high_priority(10 ** 6):
                nc.vector.tensor_copy(kt_bf[:, 0:512], lead_ps[0][:, :])
                nc.scalar.copy(qt_bf[:, 0:512], lead_ps[2][:, :])
                nc.vector.tensor_copy(kt_bf[:, 512:1024], lead_ps[1][:, :])

            def proj_k_lead(nt):
                """k-proj p0 nt (2 or 3), bf16 drain only."""
                ps = pjp.tile([128, 512], F32, tag="pj", name=f"pk0{nt}")
                for term in range(3):
                    for cp in range(4):
                        qk_mm(ps, "kh", "kl", 0, nt, term, cp)
                if with_bias:
                    qk_bias(ps, DHC, 0)
                nc.vector.tensor_copy(
                    kt_bf[:, nt * 512:(nt + 1) * 512], ps[:, :]
                )

            # --- static filler schedule -----------------------------------
            fillers = {}

            def add(wi, c, fn, front=False):
                lst = fillers.setdefault((wi, c), [])
                lst.insert(0, fn) if front else lst.append(fn)

            def add_qk(wi, c, dst, hi, lo, boff, p, nt, step=1):
                # INVARIANT: at most one other pj-tag allocation may occur
                # between the _a and _b parts (pjp ring depth 2); _b goes at
                # the front of its slot so it frees the slot before any new
                # allocation in that slot.
                add(wi, c, lambda: proj_qk_a(dst, hi, lo, p, nt))
                add(wi, c + step,
                    lambda: proj_qk_b(dst, hi, lo, boff, p, nt), front=True)

            # w0: k-nt2/3 bf16 proj, v JIT, q p0 nt1 (bf16)
            add(0, 0, lambda: proj_v(0))
            add(0, 1, lambda: proj_k_lead(2))
            add(0, 2, lambda: proj_v(1))
            add(0, 3, lambda: proj_k_lead(3))
            add(0, 4, lambda: proj_v(2))
            add(0, 4, lambda: proj_v(3))
            for t in range(4, NCH):
                add(0, t, lambda t=t: proj_v(t))
            add_qk(0, 9, None, "qh", "ql", 0, 0, 1, step=2)
            # w1: k p1 fp8 (4 tiles) + remap; q p0 nt2 (bf16)
            for nt in range(NQW):
                add_qk(1, 2 * nt, kt8, "kh", "kl", DHC, 1, nt)
            add(1, 8, lambda: remap_k(1, 0, 2))
            add(1, 10, lambda: remap_k(1, 2, NQW))
            add_qk(1, 11, None, "qh", "ql", 0, 0, 2)
            # w2: q p0 nt3 (bf16); q p1 nt0, nt1 (fp8)
            add_qk(2, 0, None, "qh", "ql", 0, 0, 3)
            add_qk(2, 4, qt8, "qh", "ql", 0, 1, 0)
            add(2, 6, lambda: remap_q(1, 0, 1))
            add_qk(2, 9, qt8, "qh", "ql", 0, 1, 1)
            add(2, 11, lambda: remap_q(1, 1, 2))
            # w3: q p1 nt2, nt3 (fp8)
            add_qk(3, 0, qt8, "qh", "ql", 0, 1, 2)
            add(3, 2, lambda: remap_q(1, 2, 3))
            add_qk(3, 5, qt8, "qh", "ql", 0, 1, 3)
            add(3, 7, lambda: remap_q(1, 3, NQW))
            # outproj: sb group 4*qw..4*qw+3 ready after window 4+qw drains
            for qw in range(NQW - 1):
                for k in range(4):
                    add(5 + qw, 3 + 3 * k, lambda sb=4 * qw + k: outproj(sb))

            # --- attention windows ----------------------------------------
            pending = []
            _dbg_last = {}

            def emit_drain_stage(c, on_act=False):
                if not pending:
                    return
                hp, qw, oacc, st = pending[0]
                if debug and c == 2:
                    _dbg_last["r"] = st["r"]
                    _dbg_last["onm"] = st["onm"]
                if c == 0:
                    st["r"] = drain_recip(hp, qw, oacc)
                    st["onm"] = [drain_norm(hp, qw, oacc, st["r"], qb, on_act)
                                 for qb in range(2)]
                elif c == 1:
                    st["onm"] += [drain_norm(hp, qw, oacc, st["r"], qb, on_act)
                                  for qb in range(2, 4)]
                elif c <= 5:
                    drain_tp(hp, qw, c - 2, st["onm"][c - 2])
                    if c == 5:
                        pending.pop(0)

            LAG = 3
            scs = {}

            def emit_qk(wi, c):
                """QK for (window wi, chunk c) -> sc tile (hoisted 1 slot
                ahead of its exp so PE fillers can't delay the ACT stream)."""
                hp, qw = wi // NQW, wi % NQW
                with tc.high_priority(10 ** 6):
                    sc = scp.tile(
                        [128, 1024], F32, tag="sc", name=f"s_{hp}{qw}{c}"
                    )
                    for i in range(2):
                        h = 2 * hp + i
                        if hp == 0:
                            nc.tensor.matmul(
                                sc[:, 512 * i:512 * (i + 1)],
                                lhsT=kt_bf[64 * i:64 * (i + 1),
                                           c * 128:(c + 1) * 128],
                                rhs=qt_bf[64 * i:64 * (i + 1),
                                          qw * 512:(qw + 1) * 512],
                                start=True,
                                stop=True,
                            )
                            continue
                        nc.tensor.matmul(
                            sc[:, 512 * i:512 * (i + 1)],
                            lhsT=kaug[:, 2 * h * S:2 * (h + 1) * S].rearrange(
                                "p (t s) -> p t s", t=2
                            )[:, :, c * 128:(c + 1) * 128],
                            rhs=qaug[:, h * S + qw * 512:
                                     h * S + (qw + 1) * 512]
                            .rearrange("p (o n) -> p o n", o=1)
                            .broadcast_to([128, 2, 512]),
                            start=True,
                            stop=True,
                            perf_mode=DR,
                        )
                scs[(wi, c)] = sc

            prev_pvs = []        # deferred PV thunks from the previous window
            emit_qk(0, 0)
            for wi in range(NW):
                hp, qw = wi // NQW, wi % NQW
                oacc = opp.tile([128, 1024], F32, tag="oacc", name=f"o_{hp}{qw}")
                pts = {}

                def emit_pv(c, oacc=oacc, hp=hp, pts=pts):
                    pt_t = pts.pop(c)
                    for qb in range(4):
                        for i in range(2):
                            j = qb * 2 + i
                            # start=True would zero the whole 2KB PSUM
                            # zero-region (bank), racing the previous window's
                            # norm reads (invisible to subtile dep tracking).
                            # The tile is DVE-memset to zero instead, so every
                            # PV accumulates with start=False.
                            nc.tensor.matmul(
                                oacc[:, OFFS[j]:OFFS[j] + 65],
                                lhsT=pt_t[:, i * 512 + qb * 128:
                                          i * 512 + (qb + 1) * 128],
                                rhs=v_sb[:, c * VP + (2 * hp + i) * (HD + 1):
                                         c * VP + (2 * hp + i) * (HD + 1) + 65],
                                start=False,
                                stop=(c == NCH - 1),
                                skip_group_check=True,
                            )

                for c in range(NCH):
                    sc = scs.pop((wi, c))
                    if c + 1 < NCH:
                        emit_qk(wi, c + 1)
                    elif wi + 1 < NW:
                        emit_qk(wi + 1, 0)
                    pt_t = ptp.tile([128, 1024], BF, tag="pt", name=f"p_{hp}{qw}{c}")
                    with tc.high_priority(10 ** 6):
                        if (wi, c) in SCHR:
                            it = shp.tile(
                                [128, 1024], I32, tag="si", name=f"i_{hp}{qw}{c}"
                            )
                            nc.vector.tensor_scalar(
                                out=it[:, :], in0=sc[:, :],
                                scalar1=SCHR_A, scalar2=SCHR_B,
                                op0=MUL, op1=ADD,
                            )
                            nc.gpsimd.tensor_copy(
                                pt_t[:, :], it[:, :].bitcast(F32)
                            )
                        else:
                            nc.scalar.activation(
                                pt_t[:, :], sc[:, :], EXP, scale=EXPSC
                            )
                    pts[c] = pt_t

                    if c == 0:
                        for f in prev_pvs:   # all trailing PVs BEFORE the
                            f()              # drain reads the old oacc
                        prev_pvs = []
                    for fn in fillers.get((wi, c), ()):
                        fn()
                    emit_drain_stage(c)
                    if c == 2:
                        nc.vector.memset(oacc[:, :], 0.0)
                    if c >= LAG:
                        emit_pv(c - LAG)

                prev_pvs = [
                    (lambda c=c, f=emit_pv: f(c))
                    for c in range(NCH - LAG, NCH)
                ]
                pending.append((hp, qw, oacc, {}))

            # --- tail: trailing PVs, last drain, last outproj group --------
            for f in prev_pvs:
                f()
            if debug:
                nc.sync.dma_start(dbg["v"][:, :], v_sb[:, :])
                nc.sync.dma_start(dbg["ktbf"][:, :], kt_bf[:, :])
                nc.sync.dma_start(dbg["qtbf"][:, :], qt_bf[:, :])
                nc.sync.dma_start(dbg["qaug"][:, :], qaug[:, :])
                nc.sync.dma_start(dbg["kaug"][:, :], kaug[:, :])
                _, _, oacc_last, _ = pending[0]
                dbg_o = pp.tile([128, 1024], F32, tag="dbgo", name="dbg_o")
                nc.vector.tensor_copy(dbg_o[:, :], oacc_last[:, :])
                nc.sync.dma_start(dbg["oacc"][:, :], dbg_o[:, :])
            for c in range(3):
                emit_drain_stage(c, on_act=True)
            sbs = list(range(4 * (NQW - 1), NCH))
            for c in range(3, 6):
                emit_drain_stage(c)
                outproj(sbs[c - 3], on_act=True)
            outproj(sbs[3], on_act=True)
            if debug:
                nc.sync.dma_start(dbg["onT"][:, :], onT[:, :])
                nc.sync.dma_start(dbg["r"][:, :], _dbg_last["r"][:, :])
                for qb in range(4):
                    nc.sync.dma_start(
                        dbg["onm"][:, qb * 128:(qb + 1) * 128],
                        _dbg_last["onm"][qb][:, :],
                    )

    nc.compile()
    return nc


def _get_nc(with_bias=False):
    if with_bias not in _nc_cache:
        _nc_cache[with_bias] = _build_bass(with_bias=with_bias)
    return _nc_cache[with_bias]


def _prepare_in_maps(x, wq, bq, wk, bk, wv, bv, wo, with_bias):
    import ml_dtypes

    f8 = ml_dtypes.float8_e4m3
    bf16 = ml_dtypes.bfloat16
    x = np.asarray(x, np.float32)
    wq, bq = np.asarray(wq, np.float32), np.asarray(bq, np.float32)
    wk, bk = np.asarray(wk, np.float32), np.asarray(bk, np.float32)
    wv, bv = np.asarray(wv, np.float32), np.asarray(bv, np.float32)
    wo = np.asarray(wo, np.float32)

    def split8(a):
        hi = a.astype(f8)
        lo = (a - hi.astype(np.float32)).astype(f8)
        return hi, lo

    xh, xl = [], []
    for b in range(B):
        h, l = split8(x[b].T)
        xh.append(np.ascontiguousarray(h))
        xl.append(np.ascontiguousarray(l))

    def wchunks(w):
        # [D, DHC] -> [128, KC*DHC] (chunk-major columns)
        return w.reshape(KC, 128, DHC).transpose(1, 0, 2).reshape(128, KC * DHC)

    in_maps = []
    for c in range(NC):
        b, j = divmod(c, HPC)
        cs = slice(DHC * j, DHC * (j + 1))
        parts = []
        for w in (wq[:, cs], wk[:, cs], wv[:, cs]):
            h, l = split8(w * WSC)
            parts += [wchunks(h), wchunks(l)]
        # order: qh, ql, kh, kl, vh, vl
        w8 = np.concatenate(parts, axis=1)
        m = {
            "xh": xh[b],
            "xl": xl[b],
            "w8": np.ascontiguousarray(w8),
            "wo_c": np.ascontiguousarray(wo[cs, :] / WSC).astype(bf16),
        }
        if with_bias:
            bias3 = np.concatenate(
                [bq[cs], bk[cs], bv[cs]]).reshape(1, 3 * DHC) * WSC
            m["bias3"] = np.ascontiguousarray(bias3.astype(bf16))
        in_maps.append(m)
    return in_maps


def kernel(x, wq, bq, wk, bk, wv, bv, wo, bo):
    from concourse import bass_utils

    with_bias = bool(
        np.any(np.asarray(bq)) or np.any(np.asarray(bk)) or np.any(np.asarray(bv))
    )
    in_maps = _prepare_in_maps(x, wq, bq, wk, bk, wv, bv, wo, with_bias)
    res = bass_utils.run_bass_kernel_spmd(
        nc=_get_nc(with_bias), in_maps=in_maps, core_ids=list(range(NC))
    )
    bo = np.asarray(bo, np.float32)
    out = np.empty((B, S, D), np.float32)
    for b in range(B):
        acc = np.asarray(res.results[HPC * b]["out"], np.float32)
        for j in range(1, HPC):
            acc = acc + np.asarray(res.results[HPC * b + j]["out"], np.float32)
        out[b] = acc + bo
    return out
